# revision 14
# baseline (speedup 1.0000x reference)
"""Bahdanau attention (B=16, T=4096, H=1024) as a Trainium2 Bass/Tile kernel.

Strategy
--------
Data-parallel over batch: 8 NeuronCores x 2 batches each, no collectives.
The big matmul (k_proj = keys @ Wk.T, 68.7 GMAC) runs in bf16 on the PE at
1 cycle/row with fp32 PSUM accumulation. Per core and per batch:
  q_proj = Wq @ q                      (once per core for both batches)
  for each T-tile of 512 keys rows:
    cast-DMA keys tile [512, 1024] fp32->bf16 (SWDGE), partition = t % 128
    PE-transpose 128x128 blocks -> keysT [h, t] chunks (bf16, 1 cyc/row)
    k_proj[g, t] = sum_h WkT[h, g]^T keysT[h, t]   (bf16 matmuls, fp32 acc)
    tanh_t = tanh(k_proj + q_proj[g])              (ACT, fused per-partition bias)
    score[1, t] = v^T tanh_t                       (PE, v stationary)
    e = exp(score)  (+ running Z via ACT accum_out; scores are bounded by
                     sum|v| <= ~16 so no max-subtraction is needed)
    attn columns [t, 1] via PE transpose of e
    ctx += e^T keys_tile                           (PE, attn stationary)
  ctx /= Z ; attn = e / Z ; DMA out

Weights (Wq, Wk, v, q) are pre-packed on the host (transpose + bf16 cast)
and replicated to all cores; keys stream in fp32 and are cast by the DMA.
"""

import ml_dtypes
import numpy as np

import concourse.bacc as bacc
import concourse.mybir as mybir
import concourse.tile as tile
from concourse.bass_utils import run_bass_kernel_spmd
from concourse.masks import make_identity

P = 128
B_FULL, T_FULL, H_FULL = 16, 4096, 1024
N_CORES = 8
USE_DMA_TRANSPOSE = True

F32 = mybir.dt.float32
BF16 = mybir.dt.bfloat16
AF = mybir.ActivationFunctionType
ALU = mybir.AluOpType
AXIS = mybir.AxisListType


def build(nc, B=B_FULL // N_CORES, T=T_FULL, H=H_FULL, TT=512):
    """Emit the per-core program: B batches, T keys rows, H features.

    The main loop is an explicit 3-stage software pipeline over T-tiles:
      step i emits: transposes(i) | score+exp(i-1) | attn-tail(i-2) | k_proj(i)
    so every PE instruction's producers ran at least one full step earlier and
    the PE stream never stalls on ACT/DVE round-trips.
    """
    G = H                 # projection dim (Wq/Wk are square)
    HC = H // P           # h-chunks of 128 (contraction)
    GC = G // P           # g-chunks of 128
    TC = TT // P          # t-chunks of 128 per T-tile
    NT = T // TT          # T-tiles per batch
    NH = H // 512         # 512-wide column groups of H

    keys_d = nc.dram_tensor("keys_in", [B, T, H], F32, kind="ExternalInput").ap()
    qT_d = nc.dram_tensor("qT_in", [H, B], BF16, kind="ExternalInput").ap()
    wkT_d = nc.dram_tensor("wkT_in", [H, G], BF16, kind="ExternalInput").ap()
    wqT_d = nc.dram_tensor("wqT_in", [H, G], BF16, kind="ExternalInput").ap()
    vP_d = nc.dram_tensor("vP_in", [P, GC], BF16, kind="ExternalInput").ap()
    ctx_d = nc.dram_tensor("ctx_out", [B, H], F32, kind="ExternalOutput").ap()
    attn_d = nc.dram_tensor("attn_out", [B, T], F32, kind="ExternalOutput").ap()

    with tile.TileContext(nc) as tc:
        with (
            tc.tile_pool(name="consts", bufs=1) as consts,
            tc.tile_pool(name="psA", bufs=2, space="PSUM") as psA,  # keys transposes
            tc.tile_pool(name="psB", bufs=3 if USE_DMA_TRANSPOSE else 2,
                         space="PSUM") as psB,  # k_proj out
            tc.tile_pool(name="psC", bufs=2, space="PSUM") as psC,  # score / attn-T
            tc.tile_pool(name="psD", bufs=2, space="PSUM") as psD,  # context partials
            tc.tile_pool(name="ph0", bufs=1) as ph0,
            tc.tile_pool(name="kpool",
                         bufs=5 if USE_DMA_TRANSPOSE else 4) as kpool,
            tc.tile_pool(name="work", bufs=2) as work,
            tc.tile_pool(name="kdpool", bufs=3, space="DRAM") as kdpool,
        ):
            identB = consts.tile([P, P], BF16)
            make_identity(nc, identB)
            identF = consts.tile([B, B], F32)
            make_identity(nc, identF)

            wkt = consts.tile([P, HC, G], BF16)  # wkt[p,hi,g] = Wk[g, hi*P+p]
            vsb = consts.tile([P, GC], BF16)     # vsb[p,gi] = v[gi*P+p]
            wqt = ph0.tile([P, HC, G], BF16)
            qt = ph0.tile([P, HC, B], BF16)      # qt[p,hi,b] = query[b, hi*P+p]

            qbias = consts.tile([P, GC, B], F32)   # q_proj, bias layout
            attn_sb = consts.tile([1, B * T], F32)  # unnormalized exp(score)
            ctx_acc = consts.tile([1, B * H], F32)  # unnormalized context
            zparts = consts.tile([1, B * NT], F32)  # per-tile exp sums
            ztot = consts.tile([1, B], F32)
            rz = consts.tile([1, B], F32)

            HGRP = min(4, HC)  # h-chunks per transpose psum tile

            def stage_load(b, ti):
                """fp32 -> bf16 cast DMA (SWDGE) of one keys tile."""
                t0 = ti * TT
                kbf = kpool.tile([P, TC, H], BF16, tag="kbf", name=f"kbf_{b}_{ti}")
                nc.gpsimd.dma_start(
                    kbf[:],
                    keys_d[b, t0:t0 + TT, :].rearrange("(a p) h -> p a h", p=P),
                )
                return kbf

            def stage_dma_transpose(b, ti, kbf):
                """keysT via bf16 DRAM round-trip + XBAR transpose-DMA."""
                kdram = kdpool.tile([TT, H], BF16, name=f"kdram_{b}_{ti}")
                nc.scalar.dma_start(
                    kdram.rearrange("(a p) h -> p a h", p=P), kbf[:]
                )
                keysT = work.tile([P, HC, TT], BF16, tag="keysT",
                                  name=f"keysT_{b}_{ti}",
                                  bufs=3 if USE_DMA_TRANSPOSE else 2)
                for hi in range(HC):
                    nc.sync.dma_start_transpose(
                        keysT[:, hi, :], kdram[:, hi * P:(hi + 1) * P]
                    )
                return keysT

            def stage_transpose(b, ti, kbf):
                """keysT[p, hi, t'] = keys[b, t0+t', hi*P+p] via PE transposes."""
                keysT = work.tile([P, HC, TT], BF16, tag="keysT",
                                  name=f"keysT_{b}_{ti}")
                for tci in range(TC):
                    for half in range(HC // HGRP):
                        tp = psA.tile([P, 512], BF16, tag="tp",
                                      name=f"tp_{b}_{ti}_{tci}_{half}")
                        for j in range(HGRP):
                            hj = half * HGRP + j
                            nc.tensor.transpose(
                                tp[:, j * P:(j + 1) * P],
                                kbf[:, tci, hj * P:(hj + 1) * P],
                                identB[:],
                            )
                        nc.vector.tensor_copy(
                            keysT[:, half * HGRP:(half + 1) * HGRP,
                                  tci * P:(tci + 1) * P],
                            tp[:, :HGRP * P].rearrange("p (a q) -> p a q", a=HGRP),
                        )
                return keysT

            def stage_kproj(b, ti, keysT):
                """k_proj chunks + tanh(. + q_proj); th[p, gi, t'] (bf16)."""
                th = work.tile([P, GC, TT], BF16, tag="th", name=f"th_{b}_{ti}")
                for gi in range(GC):
                    kp = psB.tile([P, 512], F32, tag="kp",
                                  name=f"kp_{b}_{ti}_{gi}")
                    for hi in range(HC):
                        nc.tensor.matmul(
                            kp[:, :TT],
                            wkt[:, hi, gi * P:(gi + 1) * P],
                            keysT[:, hi, :],
                            start=(hi == 0),
                            stop=(hi == HC - 1),
                        )
                    nc.scalar.activation(
                        th[:, gi, :], kp[:, :TT], AF.Tanh,
                        bias=qbias[:, gi, b:b + 1],
                    )
                return th

            def stage_score(b, ti, th):
                """score = v . th on PE, then exp (+ running partial of Z)."""
                t0 = ti * TT
                ab = b * T
                sc_t = psC.tile([P, 512], F32, tag="small", name=f"sc_{b}_{ti}")
                sc = sc_t[0:1, :TT]
                for gi in range(GC):
                    nc.tensor.matmul(
                        sc,
                        vsb[:, gi:gi + 1],
                        th[:, gi, :],
                        start=(gi == 0),
                        stop=(gi == GC - 1),
                    )
                nc.scalar.activation(
                    attn_sb[0:1, ab + t0:ab + t0 + TT],
                    sc,
                    AF.Exp,
                    accum_out=zparts[0:1, b * NT + ti:b * NT + ti + 1],
                )

            def stage_attn_tail(b, ti, kbf):
                """attn columns via PE transpose, then ctx += e^T @ keys_tile."""
                t0 = ti * TT
                ab = b * T
                at_t = psC.tile([P, 512], F32, tag="small", name=f"at_{b}_{ti}")
                for tci in range(TC):
                    nc.tensor.transpose(
                        at_t[:, tci:tci + 1],
                        attn_sb[0:1, ab + t0 + tci * P:ab + t0 + (tci + 1) * P],
                        identF[0:1, 0:1],
                    )
                acols = work.tile([P, TC], BF16, tag="acols",
                                  name=f"acols_{b}_{ti}")
                nc.vector.tensor_copy(acols[:], at_t[:, :TC])

                for nh in range(NH):
                    cx = psD.tile([1, 512], F32, tag="cx", name=f"cx_{b}_{ti}_{nh}")
                    for tci in range(TC):
                        nc.tensor.matmul(
                            cx[:],
                            acols[:, tci:tci + 1],
                            kbf[:, tci, nh * 512:(nh + 1) * 512],
                            start=(tci == 0),
                            stop=(tci == TC - 1),
                        )
                    dst = ctx_acc[0:1, b * H + nh * 512:b * H + (nh + 1) * 512]
                    if ti == 0:
                        nc.vector.tensor_copy(dst, cx[:])
                    else:
                        nc.vector.tensor_add(dst, dst, cx[:])

            def batch_end(b):
                """Normalize attn and ctx by 1/Z and DMA out."""
                ab = b * T
                nc.vector.tensor_reduce(
                    ztot[0:1, b:b + 1],
                    zparts[0:1, b * NT:(b + 1) * NT],
                    axis=AXIS.X,
                    op=ALU.add,
                )
                nc.vector.reciprocal(rz[0:1, b:b + 1], ztot[0:1, b:b + 1])
                nc.scalar.mul(
                    attn_sb[0:1, ab:ab + T], attn_sb[0:1, ab:ab + T],
                    rz[0:1, b:b + 1],
                )
                nc.sync.dma_start(attn_d[b:b + 1, :], attn_sb[0:1, ab:ab + T])
                nc.vector.tensor_scalar_mul(
                    ctx_acc[0:1, b * H:(b + 1) * H],
                    ctx_acc[0:1, b * H:(b + 1) * H],
                    rz[0:1, b:b + 1],
                )
                nc.sync.dma_start(ctx_d[b:b + 1, :], ctx_acc[0:1, b * H:(b + 1) * H])

            def phase0():
                """q_proj for both batches -> qbias."""
                qp_sb = ph0.tile([B, G], F32)    # q_proj[b, g]
                for gh in range(G // 512):
                    qp_ps = psB.tile([P, 512], F32, tag="kp", name=f"qp_ps_{gh}")
                    for hi in range(HC):
                        nc.tensor.matmul(
                            qp_ps[:B, :],
                            qt[:, hi, :],
                            wqt[:, hi, gh * 512:(gh + 1) * 512],
                            start=(hi == 0),
                            stop=(hi == HC - 1),
                        )
                    nc.scalar.copy(qp_sb[:, gh * 512:(gh + 1) * 512], qp_ps[:B, :])
                for gi in range(GC):
                    tpq = psC.tile([P, 512], F32, tag="small", name=f"tpq_{gi}")
                    nc.tensor.transpose(
                        tpq[:, :B], qp_sb[:, gi * P:(gi + 1) * P], identF[:]
                    )
                    nc.vector.tensor_copy(qbias[:, gi, :], tpq[:, :B])

            # ---------------- pipelined emission ----------------
            steps = [(b, ti) for b in range(B) for ti in range(NT)]
            n = len(steps)
            kbfs, keysTs, ths = {}, {}, {}

            # prologue. DMA issue order matters: the first keys tile goes
            # first (SWDGE, the PE's first dependency), then the main-loop
            # weights (ACT HWDGE), then the q_proj inputs (SP HWDGE) which
            # land while the PE does the first transposes.
            mk_keysT = (stage_dma_transpose if USE_DMA_TRANSPOSE
                        else stage_transpose)
            lead = 2 if USE_DMA_TRANSPOSE else 1  # keysT-chain lead over k_proj

            kbfs[0] = stage_load(*steps[0])
            nc.scalar.dma_start(wkt[:], wkT_d.rearrange("(a p) g -> p a g", p=P))
            nc.sync.dma_start(wqt[:], wqT_d.rearrange("(a p) g -> p a g", p=P))
            nc.sync.dma_start(qt[:], qT_d.rearrange("(a p) b -> p a b", p=P))
            nc.scalar.dma_start(vsb[:], vP_d)
            keysTs[0] = mk_keysT(*steps[0], kbfs[0])
            for i in range(1, min(lead, n)):
                kbfs[i] = stage_load(*steps[i])
                keysTs[i] = mk_keysT(*steps[i], kbfs[i])
            phase0()

            for i in range(n + 2):
                if lead <= i < n:
                    kbfs[i] = stage_load(*steps[i])
                    keysTs[i] = mk_keysT(*steps[i], kbfs[i])
                if 0 <= i - 1 < n:
                    j = i - 1
                    stage_score(*steps[j], ths.pop(j))
                if 0 <= i - 2 < n:
                    j = i - 2
                    stage_attn_tail(*steps[j], kbfs.pop(j))
                    b, ti = steps[j]
                    if ti == NT - 1:
                        batch_end(b)
                if i < n:
                    ths[i] = stage_kproj(*steps[i], keysTs.pop(i))


_compiled = None

# test-harness knobs (the grading harness uses the defaults)
TRACE = False
LAST_RESULT = None


def _get_nc():
    global _compiled
    if _compiled is None:
        nc = bacc.Bacc(
            "TRN2",
            target_bir_lowering=False,
            debug=False,
            enable_asserts=False,
            num_devices=N_CORES,
        )
        build(nc)
        nc.compile()
        _compiled = nc
    return _compiled


def kernel(query, keys, Wq, Wk, v):
    query = np.asarray(query, dtype=np.float32)
    keys = np.asarray(keys, dtype=np.float32)
    Wq = np.asarray(Wq, dtype=np.float32)
    Wk = np.asarray(Wk, dtype=np.float32)
    v = np.asarray(v, dtype=np.float32)

    B = B_FULL // N_CORES
    GC = H_FULL // P
    bf = ml_dtypes.bfloat16
    # host-side weight pre-packing (transpose + bf16 cast), replicated per core
    wkT = np.ascontiguousarray(Wk.T).astype(bf)
    wqT = np.ascontiguousarray(Wq.T).astype(bf)
    vP = np.ascontiguousarray(v.reshape(GC, P).T).astype(bf)

    nc = _get_nc()
    in_maps = []
    for c in range(N_CORES):
        qs = query[c * B:(c + 1) * B, 0, :]  # [B, H]
        in_maps.append({
            "keys_in": np.ascontiguousarray(keys[c * B:(c + 1) * B]),
            "qT_in": np.ascontiguousarray(qs.T).astype(bf),
            "wkT_in": wkT,
            "wqT_in": wqT,
            "vP_in": vP,
        })

    global LAST_RESULT
    res = run_bass_kernel_spmd(
        nc, in_maps, core_ids=list(range(N_CORES)), trace=TRACE
    )
    LAST_RESULT = res
    ctx = np.concatenate([r["ctx_out"] for r in res.results], axis=0)[:, None, :]
    attn = np.concatenate([r["attn_out"] for r in res.results], axis=0)[:, None, :]
    return ctx, attn


# revision 15
# speedup vs baseline: 1.3792x; 1.3792x over previous
"""Bahdanau attention (B=16, T=4096, H=1024) as a Trainium2 Bass/Tile kernel.

Strategy
--------
Data-parallel over batch: 8 NeuronCores x 2 batches each, no collectives.
The big matmul (k_proj = keys @ Wk.T, 68.7 GMAC) runs in bf16 on the PE at
1 cycle/row with fp32 PSUM accumulation. Per core and per batch:
  q_proj = Wq @ q                      (once per core for both batches)
  for each T-tile of 512 keys rows:
    cast-DMA keys tile [512, 1024] fp32->bf16 (SWDGE), partition = t % 128
    PE-transpose 128x128 blocks -> keysT [h, t] chunks (bf16, 1 cyc/row)
    k_proj[g, t] = sum_h WkT[h, g]^T keysT[h, t]   (bf16 matmuls, fp32 acc)
    tanh_t = tanh(k_proj + q_proj[g])              (ACT, fused per-partition bias)
    score[1, t] = v^T tanh_t                       (PE, v stationary)
    e = exp(score)  (+ running Z via ACT accum_out; scores are bounded by
                     sum|v| <= ~16 so no max-subtraction is needed)
    attn columns [t, 1] via PE transpose of e
    ctx += e^T keys_tile                           (PE, attn stationary)
  ctx /= Z ; attn = e / Z ; DMA out

Weights (Wq, Wk, v, q) are pre-packed on the host (transpose + bf16 cast)
and replicated to all cores; keys stream in fp32 and are cast by the DMA.
"""

import ml_dtypes
import numpy as np

import concourse.bacc as bacc
import concourse.mybir as mybir
import concourse.tile as tile
from concourse.bass_utils import run_bass_kernel_spmd
from concourse.masks import make_identity

P = 128
B_FULL, T_FULL, H_FULL = 16, 4096, 1024
N_CORES = 8
USE_DMA_TRANSPOSE = False

F32 = mybir.dt.float32
BF16 = mybir.dt.bfloat16
AF = mybir.ActivationFunctionType
ALU = mybir.AluOpType
AXIS = mybir.AxisListType


def build(nc, B=B_FULL // N_CORES, T=T_FULL, H=H_FULL, TT=512):
    """Emit the per-core program: B batches, T keys rows, H features.

    The main loop is an explicit 3-stage software pipeline over T-tiles:
      step i emits: transposes(i) | score+exp(i-1) | attn-tail(i-2) | k_proj(i)
    so every PE instruction's producers ran at least one full step earlier and
    the PE stream never stalls on ACT/DVE round-trips.
    """
    G = H                 # projection dim (Wq/Wk are square)
    HC = H // P           # h-chunks of 128 (contraction)
    GC = G // P           # g-chunks of 128
    TC = TT // P          # t-chunks of 128 per T-tile
    NT = T // TT          # T-tiles per batch
    NH = H // 512         # 512-wide column groups of H

    keys_d = nc.dram_tensor("keys_in", [B, T, H], F32, kind="ExternalInput").ap()
    qT_d = nc.dram_tensor("qT_in", [H, B], BF16, kind="ExternalInput").ap()
    wkT_d = nc.dram_tensor("wkT_in", [H, G], BF16, kind="ExternalInput").ap()
    wqT_d = nc.dram_tensor("wqT_in", [H, G], BF16, kind="ExternalInput").ap()
    vP_d = nc.dram_tensor("vP_in", [P, GC], BF16, kind="ExternalInput").ap()
    ctx_d = nc.dram_tensor("ctx_out", [B, H], F32, kind="ExternalOutput").ap()
    attn_d = nc.dram_tensor("attn_out", [B, T], F32, kind="ExternalOutput").ap()

    with tile.TileContext(nc) as tc:
        with (
            tc.tile_pool(name="consts", bufs=1) as consts,
            tc.tile_pool(name="psA", bufs=2, space="PSUM") as psA,  # keys transposes
            tc.tile_pool(name="psB", bufs=3 if USE_DMA_TRANSPOSE else 2,
                         space="PSUM") as psB,  # k_proj out
            tc.tile_pool(name="psC", bufs=2, space="PSUM") as psC,  # score / attn-T
            tc.tile_pool(name="psD", bufs=2, space="PSUM") as psD,  # context partials
            tc.tile_pool(name="ph0", bufs=1) as ph0,
            tc.tile_pool(name="kpool",
                         bufs=5 if USE_DMA_TRANSPOSE else 4) as kpool,
            tc.tile_pool(name="work", bufs=2) as work,
            tc.tile_pool(name="kdpool", bufs=3, space="DRAM") as kdpool,
        ):
            identB = consts.tile([P, P], BF16)
            make_identity(nc, identB)
            identF = consts.tile([B, B], F32)
            make_identity(nc, identF)

            wkt = consts.tile([P, HC, G], BF16)  # wkt[p,hi,g] = Wk[g, hi*P+p]
            vsb = consts.tile([P, GC], BF16)     # vsb[p,gi] = v[gi*P+p]
            wqt = ph0.tile([P, HC, G], BF16)
            qt = ph0.tile([P, HC, B], BF16)      # qt[p,hi,b] = query[b, hi*P+p]

            qbias = consts.tile([P, GC, B], F32)   # q_proj, bias layout
            attn_sb = consts.tile([1, B * T], F32)  # unnormalized exp(score)
            ctx_acc = consts.tile([1, B * H], F32)  # unnormalized context
            zparts = consts.tile([1, B * NT], F32)  # per-tile exp sums
            ztot = consts.tile([1, B], F32)
            rz = consts.tile([1, B], F32)

            HGRP = min(4, HC)  # h-chunks per transpose psum tile

            def stage_load(b, ti):
                """fp32 -> bf16 cast DMA (SWDGE) of one keys tile."""
                t0 = ti * TT
                kbf = kpool.tile([P, TC, H], BF16, tag="kbf", name=f"kbf_{b}_{ti}")
                nc.gpsimd.dma_start(
                    kbf[:],
                    keys_d[b, t0:t0 + TT, :].rearrange("(a p) h -> p a h", p=P),
                )
                return kbf

            def stage_dma_transpose(b, ti, kbf):
                """keysT via bf16 DRAM round-trip + XBAR transpose-DMA."""
                kdram = kdpool.tile([TT, H], BF16, name=f"kdram_{b}_{ti}")
                nc.scalar.dma_start(
                    kdram.rearrange("(a p) h -> p a h", p=P), kbf[:]
                )
                keysT = work.tile([P, HC, TT], BF16, tag="keysT",
                                  name=f"keysT_{b}_{ti}",
                                  bufs=3 if USE_DMA_TRANSPOSE else 2)
                for hi in range(HC):
                    nc.sync.dma_start_transpose(
                        keysT[:, hi, :], kdram[:, hi * P:(hi + 1) * P]
                    )
                return keysT

            def stage_transpose(b, ti, kbf):
                """keysT[p, hi, t'] = keys[b, t0+t', hi*P+p] via PE transposes."""
                keysT = work.tile([P, HC, TT], BF16, tag="keysT",
                                  name=f"keysT_{b}_{ti}")
                for tci in range(TC):
                    for half in range(HC // HGRP):
                        tp = psA.tile([P, 512], BF16, tag="tp",
                                      name=f"tp_{b}_{ti}_{tci}_{half}")
                        for j in range(HGRP):
                            hj = half * HGRP + j
                            nc.tensor.transpose(
                                tp[:, j * P:(j + 1) * P],
                                kbf[:, tci, hj * P:(hj + 1) * P],
                                identB[:],
                            )
                        nc.vector.tensor_copy(
                            keysT[:, half * HGRP:(half + 1) * HGRP,
                                  tci * P:(tci + 1) * P],
                            tp[:, :HGRP * P].rearrange("p (a q) -> p a q", a=HGRP),
                        )
                return keysT

            def stage_kproj(b, ti, keysT):
                """k_proj chunks + tanh(. + q_proj); th[p, gi, t'] (bf16)."""
                th = work.tile([P, GC, TT], BF16, tag="th", name=f"th_{b}_{ti}")
                for gi in range(GC):
                    kp = psB.tile([P, 512], F32, tag="kp",
                                  name=f"kp_{b}_{ti}_{gi}")
                    for hi in range(HC):
                        nc.tensor.matmul(
                            kp[:, :TT],
                            wkt[:, hi, gi * P:(gi + 1) * P],
                            keysT[:, hi, :],
                            start=(hi == 0),
                            stop=(hi == HC - 1),
                        )
                    nc.scalar.activation(
                        th[:, gi, :], kp[:, :TT], AF.Tanh,
                        bias=qbias[:, gi, b:b + 1],
                    )
                return th

            def stage_score(b, ti, th):
                """score = v . th on PE, then exp (+ running partial of Z)."""
                t0 = ti * TT
                ab = b * T
                sc_t = psC.tile([P, 512], F32, tag="small", name=f"sc_{b}_{ti}")
                sc = sc_t[0:1, :TT]
                for gi in range(GC):
                    nc.tensor.matmul(
                        sc,
                        vsb[:, gi:gi + 1],
                        th[:, gi, :],
                        start=(gi == 0),
                        stop=(gi == GC - 1),
                    )
                nc.scalar.activation(
                    attn_sb[0:1, ab + t0:ab + t0 + TT],
                    sc,
                    AF.Exp,
                    accum_out=zparts[0:1, b * NT + ti:b * NT + ti + 1],
                )

            def stage_attn_tail(b, ti, kbf):
                """attn columns via PE transpose, then ctx += e^T @ keys_tile."""
                t0 = ti * TT
                ab = b * T
                at_t = psC.tile([P, 512], F32, tag="small", name=f"at_{b}_{ti}")
                for tci in range(TC):
                    nc.tensor.transpose(
                        at_t[:, tci:tci + 1],
                        attn_sb[0:1, ab + t0 + tci * P:ab + t0 + (tci + 1) * P],
                        identF[0:1, 0:1],
                    )
                acols = work.tile([P, TC], BF16, tag="acols",
                                  name=f"acols_{b}_{ti}")
                nc.vector.tensor_copy(acols[:], at_t[:, :TC])

                for nh in range(NH):
                    cx = psD.tile([1, 512], F32, tag="cx", name=f"cx_{b}_{ti}_{nh}")
                    for tci in range(TC):
                        nc.tensor.matmul(
                            cx[:],
                            acols[:, tci:tci + 1],
                            kbf[:, tci, nh * 512:(nh + 1) * 512],
                            start=(tci == 0),
                            stop=(tci == TC - 1),
                        )
                    dst = ctx_acc[0:1, b * H + nh * 512:b * H + (nh + 1) * 512]
                    if ti == 0:
                        nc.vector.tensor_copy(dst, cx[:])
                    else:
                        nc.vector.tensor_add(dst, dst, cx[:])

            def batch_end(b):
                """Normalize attn and ctx by 1/Z and DMA out."""
                ab = b * T
                nc.vector.tensor_reduce(
                    ztot[0:1, b:b + 1],
                    zparts[0:1, b * NT:(b + 1) * NT],
                    axis=AXIS.X,
                    op=ALU.add,
                )
                nc.vector.reciprocal(rz[0:1, b:b + 1], ztot[0:1, b:b + 1])
                nc.scalar.mul(
                    attn_sb[0:1, ab:ab + T], attn_sb[0:1, ab:ab + T],
                    rz[0:1, b:b + 1],
                )
                nc.sync.dma_start(attn_d[b:b + 1, :], attn_sb[0:1, ab:ab + T])
                nc.vector.tensor_scalar_mul(
                    ctx_acc[0:1, b * H:(b + 1) * H],
                    ctx_acc[0:1, b * H:(b + 1) * H],
                    rz[0:1, b:b + 1],
                )
                nc.sync.dma_start(ctx_d[b:b + 1, :], ctx_acc[0:1, b * H:(b + 1) * H])

            def phase0():
                """q_proj for both batches -> qbias."""
                qp_sb = ph0.tile([B, G], F32)    # q_proj[b, g]
                for gh in range(G // 512):
                    qp_ps = psB.tile([P, 512], F32, tag="kp", name=f"qp_ps_{gh}")
                    for hi in range(HC):
                        nc.tensor.matmul(
                            qp_ps[:B, :],
                            qt[:, hi, :],
                            wqt[:, hi, gh * 512:(gh + 1) * 512],
                            start=(hi == 0),
                            stop=(hi == HC - 1),
                        )
                    nc.scalar.copy(qp_sb[:, gh * 512:(gh + 1) * 512], qp_ps[:B, :])
                for gi in range(GC):
                    tpq = psC.tile([P, 512], F32, tag="small", name=f"tpq_{gi}")
                    nc.tensor.transpose(
                        tpq[:, :B], qp_sb[:, gi * P:(gi + 1) * P], identF[:]
                    )
                    nc.vector.tensor_copy(qbias[:, gi, :], tpq[:, :B])

            # ---------------- pipelined emission ----------------
            steps = [(b, ti) for b in range(B) for ti in range(NT)]
            n = len(steps)
            kbfs, keysTs, ths = {}, {}, {}

            # prologue. DMA issue order matters: the first keys tile goes
            # first (SWDGE, the PE's first dependency), then the main-loop
            # weights (ACT HWDGE), then the q_proj inputs (SP HWDGE) which
            # land while the PE does the first transposes.
            mk_keysT = (stage_dma_transpose if USE_DMA_TRANSPOSE
                        else stage_transpose)
            lead = 2 if USE_DMA_TRANSPOSE else 1  # keysT-chain lead over k_proj

            kbfs[0] = stage_load(*steps[0])
            nc.scalar.dma_start(wkt[:], wkT_d.rearrange("(a p) g -> p a g", p=P))
            nc.sync.dma_start(wqt[:], wqT_d.rearrange("(a p) g -> p a g", p=P))
            nc.sync.dma_start(qt[:], qT_d.rearrange("(a p) b -> p a b", p=P))
            nc.scalar.dma_start(vsb[:], vP_d)
            keysTs[0] = mk_keysT(*steps[0], kbfs[0])
            for i in range(1, min(lead, n)):
                kbfs[i] = stage_load(*steps[i])
                keysTs[i] = mk_keysT(*steps[i], kbfs[i])
            phase0()

            for i in range(n + 2):
                if lead <= i < n:
                    kbfs[i] = stage_load(*steps[i])
                    keysTs[i] = mk_keysT(*steps[i], kbfs[i])
                if 0 <= i - 1 < n:
                    j = i - 1
                    stage_score(*steps[j], ths.pop(j))
                if 0 <= i - 2 < n:
                    j = i - 2
                    stage_attn_tail(*steps[j], kbfs.pop(j))
                    b, ti = steps[j]
                    if ti == NT - 1:
                        batch_end(b)
                if i < n:
                    ths[i] = stage_kproj(*steps[i], keysTs.pop(i))


_compiled = None

# test-harness knobs (the grading harness uses the defaults)
TRACE = False
LAST_RESULT = None


def _get_nc():
    global _compiled
    if _compiled is None:
        nc = bacc.Bacc(
            "TRN2",
            target_bir_lowering=False,
            debug=False,
            enable_asserts=False,
            num_devices=N_CORES,
        )
        build(nc)
        nc.compile()
        _compiled = nc
    return _compiled


def kernel(query, keys, Wq, Wk, v):
    query = np.asarray(query, dtype=np.float32)
    keys = np.asarray(keys, dtype=np.float32)
    Wq = np.asarray(Wq, dtype=np.float32)
    Wk = np.asarray(Wk, dtype=np.float32)
    v = np.asarray(v, dtype=np.float32)

    B = B_FULL // N_CORES
    GC = H_FULL // P
    bf = ml_dtypes.bfloat16
    # host-side weight pre-packing (transpose + bf16 cast), replicated per core
    wkT = np.ascontiguousarray(Wk.T).astype(bf)
    wqT = np.ascontiguousarray(Wq.T).astype(bf)
    vP = np.ascontiguousarray(v.reshape(GC, P).T).astype(bf)

    nc = _get_nc()
    in_maps = []
    for c in range(N_CORES):
        qs = query[c * B:(c + 1) * B, 0, :]  # [B, H]
        in_maps.append({
            "keys_in": np.ascontiguousarray(keys[c * B:(c + 1) * B]),
            "qT_in": np.ascontiguousarray(qs.T).astype(bf),
            "wkT_in": wkT,
            "wqT_in": wqT,
            "vP_in": vP,
        })

    global LAST_RESULT
    res = run_bass_kernel_spmd(
        nc, in_maps, core_ids=list(range(N_CORES)), trace=TRACE
    )
    LAST_RESULT = res
    ctx = np.concatenate([r["ctx_out"] for r in res.results], axis=0)[:, None, :]
    attn = np.concatenate([r["attn_out"] for r in res.results], axis=0)[:, None, :]
    return ctx, attn


# revision 16
# speedup vs baseline: 1.4393x; 1.0436x over previous
"""Bahdanau attention (B=16, T=4096, H=1024) as a Trainium2 Bass/Tile kernel.

Strategy
--------
Data-parallel over batch: 8 NeuronCores x 2 batches each, no collectives.
The big matmul (k_proj = keys @ Wk.T, 68.7 GMAC) runs in bf16 on the PE at
1 cycle/row with fp32 PSUM accumulation. Per core and per batch:
  q_proj = Wq @ q                      (once per core for both batches)
  for each T-tile of 512 keys rows:
    cast-DMA keys tile [512, 1024] fp32->bf16 (SWDGE), partition = t % 128
    PE-transpose 128x128 blocks -> keysT [h, t] chunks (bf16, 1 cyc/row)
    k_proj[g, t] = sum_h WkT[h, g]^T keysT[h, t]   (bf16 matmuls, fp32 acc)
    tanh_t = tanh(k_proj + q_proj[g])              (ACT, fused per-partition bias)
    score[1, t] = v^T tanh_t                       (PE, v stationary)
    e = exp(score)  (+ running Z via ACT accum_out; scores are bounded by
                     sum|v| <= ~16 so no max-subtraction is needed)
    attn columns [t, 1] via PE transpose of e
    ctx += e^T keys_tile                           (PE, attn stationary)
  ctx /= Z ; attn = e / Z ; DMA out

Weights (Wq, Wk, v, q) are pre-packed on the host (transpose + bf16 cast)
and replicated to all cores; keys stream in fp32 and are cast by the DMA.
"""

import ml_dtypes
import numpy as np

import concourse.bacc as bacc
import concourse.mybir as mybir
import concourse.tile as tile
from concourse.bass_utils import run_bass_kernel_spmd
from concourse.masks import make_identity

P = 128
B_FULL, T_FULL, H_FULL = 16, 4096, 1024
N_CORES = 8
USE_DMA_TRANSPOSE = False

F32 = mybir.dt.float32
BF16 = mybir.dt.bfloat16
AF = mybir.ActivationFunctionType
ALU = mybir.AluOpType
AXIS = mybir.AxisListType


def build(nc, B=B_FULL // N_CORES, T=T_FULL, H=H_FULL, TT=512):
    """Emit the per-core program: B batches, T keys rows, H features.

    The main loop is an explicit 3-stage software pipeline over T-tiles:
      step i emits: transposes(i) | score+exp(i-1) | attn-tail(i-2) | k_proj(i)
    so every PE instruction's producers ran at least one full step earlier and
    the PE stream never stalls on ACT/DVE round-trips.
    """
    G = H                 # projection dim (Wq/Wk are square)
    HC = H // P           # h-chunks of 128 (contraction)
    GC = G // P           # g-chunks of 128
    TC = TT // P          # t-chunks of 128 per T-tile
    NT = T // TT          # T-tiles per batch
    NH = H // 512         # 512-wide column groups of H

    keys_d = nc.dram_tensor("keys_in", [B, T, H], F32, kind="ExternalInput").ap()
    qT_d = nc.dram_tensor("qT_in", [H, B], BF16, kind="ExternalInput").ap()
    wkT_d = nc.dram_tensor("wkT_in", [H, G], BF16, kind="ExternalInput").ap()
    wqT_d = nc.dram_tensor("wqT_in", [H, G], BF16, kind="ExternalInput").ap()
    vP_d = nc.dram_tensor("vP_in", [P, GC], BF16, kind="ExternalInput").ap()
    ctx_d = nc.dram_tensor("ctx_out", [B, H], F32, kind="ExternalOutput").ap()
    attn_d = nc.dram_tensor("attn_out", [B, T], F32, kind="ExternalOutput").ap()

    with tile.TileContext(nc) as tc:
        with (
            tc.tile_pool(name="consts", bufs=1) as consts,
            tc.tile_pool(name="psA", bufs=2, space="PSUM") as psA,  # keys transposes
            tc.tile_pool(name="psB", bufs=3 if USE_DMA_TRANSPOSE else 2,
                         space="PSUM") as psB,  # k_proj out
            tc.tile_pool(name="psC", bufs=2, space="PSUM") as psC,  # score / attn-T
            tc.tile_pool(name="psD", bufs=2, space="PSUM") as psD,  # context partials
            tc.tile_pool(name="ph0", bufs=1) as ph0,
            tc.tile_pool(name="kpool",
                         bufs=5 if USE_DMA_TRANSPOSE else 4) as kpool,
            tc.tile_pool(name="work", bufs=2) as work,
            tc.tile_pool(name="kdpool", bufs=3, space="DRAM") as kdpool,
        ):
            identB = consts.tile([P, P], BF16)
            make_identity(nc, identB)
            identF = consts.tile([B, B], F32)
            make_identity(nc, identF)

            wkt = consts.tile([P, HC, G], BF16)  # wkt[p,hi,g] = Wk[g, hi*P+p]
            vsb = consts.tile([P, GC], BF16)     # vsb[p,gi] = v[gi*P+p]
            wqt = ph0.tile([P, HC, G], BF16)
            qt = ph0.tile([P, HC, B], BF16)      # qt[p,hi,b] = query[b, hi*P+p]

            qbias = consts.tile([P, GC, B], F32)   # q_proj, bias layout
            attn_sb = consts.tile([1, B * T], F32)  # unnormalized exp(score)
            ctx_acc = consts.tile([1, B * H], F32)  # unnormalized context
            zparts = consts.tile([1, B * NT], F32)  # per-tile exp sums
            ztot = consts.tile([1, B], F32)
            rz = consts.tile([1, B], F32)

            HGRP = min(4, HC)  # h-chunks per transpose psum tile

            def stage_load(b, ti):
                """fp32 -> bf16 cast DMA (SWDGE) of one keys tile."""
                t0 = ti * TT
                kbf = kpool.tile([P, TC, H], BF16, tag="kbf", name=f"kbf_{b}_{ti}")
                nc.gpsimd.dma_start(
                    kbf[:],
                    keys_d[b, t0:t0 + TT, :].rearrange("(a p) h -> p a h", p=P),
                )
                return kbf

            def stage_dma_transpose(b, ti, kbf):
                """keysT via bf16 DRAM round-trip + XBAR transpose-DMA."""
                kdram = kdpool.tile([TT, H], BF16, name=f"kdram_{b}_{ti}")
                nc.scalar.dma_start(
                    kdram.rearrange("(a p) h -> p a h", p=P), kbf[:]
                )
                keysT = work.tile([P, HC, TT], BF16, tag="keysT",
                                  name=f"keysT_{b}_{ti}",
                                  bufs=3 if USE_DMA_TRANSPOSE else 2)
                for hi in range(HC):
                    nc.sync.dma_start_transpose(
                        keysT[:, hi, :], kdram[:, hi * P:(hi + 1) * P]
                    )
                return keysT

            def stage_transpose(b, ti, kbf):
                """keysT[p, hi, t'] = keys[b, t0+t', hi*P+p] via PE transposes."""
                keysT = work.tile([P, HC, TT], BF16, tag="keysT",
                                  name=f"keysT_{b}_{ti}")
                for tci in range(TC):
                    for half in range(HC // HGRP):
                        tp = psA.tile([P, 512], BF16, tag="tp",
                                      name=f"tp_{b}_{ti}_{tci}_{half}")
                        for j in range(HGRP):
                            hj = half * HGRP + j
                            nc.tensor.transpose(
                                tp[:, j * P:(j + 1) * P],
                                kbf[:, tci, hj * P:(hj + 1) * P],
                                identB[:],
                            )
                        nc.vector.tensor_copy(
                            keysT[:, half * HGRP:(half + 1) * HGRP,
                                  tci * P:(tci + 1) * P],
                            tp[:, :HGRP * P].rearrange("p (a q) -> p a q", a=HGRP),
                        )
                return keysT

            def stage_kproj(b, ti, keysT):
                """k_proj chunks + tanh(. + q_proj); th[p, gi, t'] (bf16)."""
                th = work.tile([P, GC, TT], BF16, tag="th", name=f"th_{b}_{ti}")
                for gi in range(GC):
                    kp = psB.tile([P, 512], F32, tag="kp",
                                  name=f"kp_{b}_{ti}_{gi}")
                    for hi in range(HC):
                        nc.tensor.matmul(
                            kp[:, :TT],
                            wkt[:, hi, gi * P:(gi + 1) * P],
                            keysT[:, hi, :],
                            start=(hi == 0),
                            stop=(hi == HC - 1),
                        )
                    nc.scalar.activation(
                        th[:, gi, :], kp[:, :TT], AF.Tanh,
                        bias=qbias[:, gi, b:b + 1],
                    )
                return th

            def stage_score(b, ti, th):
                """score = v . th on PE, then exp (+ running partial of Z)."""
                t0 = ti * TT
                ab = b * T
                sc_t = psC.tile([P, 512], F32, tag="small", name=f"sc_{b}_{ti}")
                sc = sc_t[0:1, :TT]
                for gi in range(GC):
                    nc.tensor.matmul(
                        sc,
                        vsb[:, gi:gi + 1],
                        th[:, gi, :],
                        start=(gi == 0),
                        stop=(gi == GC - 1),
                    )
                nc.scalar.activation(
                    attn_sb[0:1, ab + t0:ab + t0 + TT],
                    sc,
                    AF.Exp,
                    accum_out=zparts[0:1, b * NT + ti:b * NT + ti + 1],
                )

            def stage_attn_tail(b, ti, kbf):
                """attn columns via PE transpose, then ctx += e^T @ keys_tile."""
                t0 = ti * TT
                ab = b * T
                at_t = psC.tile([P, 512], F32, tag="small", name=f"at_{b}_{ti}")
                for tci in range(TC):
                    nc.tensor.transpose(
                        at_t[:, tci:tci + 1],
                        attn_sb[0:1, ab + t0 + tci * P:ab + t0 + (tci + 1) * P],
                        identF[0:1, 0:1],
                    )
                acols = work.tile([P, TC], BF16, tag="acols",
                                  name=f"acols_{b}_{ti}")
                nc.vector.tensor_copy(acols[:], at_t[:, :TC])

                for nh in range(NH):
                    cx = psD.tile([1, 512], F32, tag="cx", name=f"cx_{b}_{ti}_{nh}")
                    for tci in range(TC):
                        nc.tensor.matmul(
                            cx[:],
                            acols[:, tci:tci + 1],
                            kbf[:, tci, nh * 512:(nh + 1) * 512],
                            start=(tci == 0),
                            stop=(tci == TC - 1),
                        )
                    dst = ctx_acc[0:1, b * H + nh * 512:b * H + (nh + 1) * 512]
                    if ti == 0:
                        nc.vector.tensor_copy(dst, cx[:])
                    else:
                        nc.vector.tensor_add(dst, dst, cx[:])

            def batch_end(b):
                """Normalize attn and ctx by 1/Z and DMA out."""
                ab = b * T
                nc.vector.tensor_reduce(
                    ztot[0:1, b:b + 1],
                    zparts[0:1, b * NT:(b + 1) * NT],
                    axis=AXIS.X,
                    op=ALU.add,
                )
                nc.vector.reciprocal(rz[0:1, b:b + 1], ztot[0:1, b:b + 1])
                nc.scalar.mul(
                    attn_sb[0:1, ab:ab + T], attn_sb[0:1, ab:ab + T],
                    rz[0:1, b:b + 1],
                )
                nc.sync.dma_start(attn_d[b:b + 1, :], attn_sb[0:1, ab:ab + T])
                nc.vector.tensor_scalar_mul(
                    ctx_acc[0:1, b * H:(b + 1) * H],
                    ctx_acc[0:1, b * H:(b + 1) * H],
                    rz[0:1, b:b + 1],
                )
                nc.sync.dma_start(ctx_d[b:b + 1, :], ctx_acc[0:1, b * H:(b + 1) * H])

            def phase0():
                """q_proj for both batches -> qbias."""
                qp_sb = ph0.tile([B, G], F32)    # q_proj[b, g]
                for gh in range(G // 512):
                    qp_ps = psB.tile([P, 512], F32, tag="kp", name=f"qp_ps_{gh}")
                    for hi in range(HC):
                        nc.tensor.matmul(
                            qp_ps[:B, :],
                            qt[:, hi, :],
                            wqt[:, hi, gh * 512:(gh + 1) * 512],
                            start=(hi == 0),
                            stop=(hi == HC - 1),
                        )
                    nc.scalar.copy(qp_sb[:, gh * 512:(gh + 1) * 512], qp_ps[:B, :])
                for gi in range(GC):
                    tpq = psC.tile([P, 512], F32, tag="small", name=f"tpq_{gi}")
                    nc.tensor.transpose(
                        tpq[:, :B], qp_sb[:, gi * P:(gi + 1) * P], identF[:]
                    )
                    nc.vector.tensor_copy(qbias[:, gi, :], tpq[:, :B])

            # ---------------- pipelined emission ----------------
            steps = [(b, ti) for b in range(B) for ti in range(NT)]
            n = len(steps)
            kbfs, keysTs, ths = {}, {}, {}

            # prologue. DMA issue order matters: the first keys tile goes
            # first (SWDGE, the PE's first dependency), then the main-loop
            # weights (ACT HWDGE), then the q_proj inputs (SP HWDGE) which
            # land while the PE does the first transposes.
            mk_keysT = (stage_dma_transpose if USE_DMA_TRANSPOSE
                        else stage_transpose)
            lead = 2 if USE_DMA_TRANSPOSE else 1  # keysT-chain lead over k_proj

            # All startup loads go through the single SWDGE queue so they
            # execute strictly in this order instead of round-robining on the
            # SDMA engines: the first keys tile is the PE's first dependency,
            # then the q_proj inputs (q_proj runs while wkt streams in), then
            # Wk for the first k_proj.
            kbfs[0] = stage_load(*steps[0])
            nc.gpsimd.dma_start(wqt[:], wqT_d.rearrange("(a p) g -> p a g", p=P))
            nc.gpsimd.dma_start(qt[:], qT_d.rearrange("(a p) b -> p a b", p=P))
            nc.gpsimd.dma_start(wkt[:], wkT_d.rearrange("(a p) g -> p a g", p=P))
            nc.gpsimd.dma_start(vsb[:], vP_d)
            keysTs[0] = mk_keysT(*steps[0], kbfs[0])
            for i in range(1, min(lead, n)):
                kbfs[i] = stage_load(*steps[i])
                keysTs[i] = mk_keysT(*steps[i], kbfs[i])
            phase0()

            for i in range(n + 2):
                if lead <= i < n:
                    kbfs[i] = stage_load(*steps[i])
                    keysTs[i] = mk_keysT(*steps[i], kbfs[i])
                if 0 <= i - 1 < n:
                    j = i - 1
                    stage_score(*steps[j], ths.pop(j))
                if 0 <= i - 2 < n:
                    j = i - 2
                    stage_attn_tail(*steps[j], kbfs.pop(j))
                    b, ti = steps[j]
                    if ti == NT - 1:
                        batch_end(b)
                if i < n:
                    ths[i] = stage_kproj(*steps[i], keysTs.pop(i))


_compiled = None

# test-harness knobs (the grading harness uses the defaults)
TRACE = False
LAST_RESULT = None


def _get_nc():
    global _compiled
    if _compiled is None:
        nc = bacc.Bacc(
            "TRN2",
            target_bir_lowering=False,
            debug=False,
            enable_asserts=False,
            num_devices=N_CORES,
        )
        build(nc)
        nc.compile()
        _compiled = nc
    return _compiled


def kernel(query, keys, Wq, Wk, v):
    query = np.asarray(query, dtype=np.float32)
    keys = np.asarray(keys, dtype=np.float32)
    Wq = np.asarray(Wq, dtype=np.float32)
    Wk = np.asarray(Wk, dtype=np.float32)
    v = np.asarray(v, dtype=np.float32)

    B = B_FULL // N_CORES
    GC = H_FULL // P
    bf = ml_dtypes.bfloat16
    # host-side weight pre-packing (transpose + bf16 cast), replicated per core
    wkT = np.ascontiguousarray(Wk.T).astype(bf)
    wqT = np.ascontiguousarray(Wq.T).astype(bf)
    vP = np.ascontiguousarray(v.reshape(GC, P).T).astype(bf)

    nc = _get_nc()
    in_maps = []
    for c in range(N_CORES):
        qs = query[c * B:(c + 1) * B, 0, :]  # [B, H]
        in_maps.append({
            "keys_in": np.ascontiguousarray(keys[c * B:(c + 1) * B]),
            "qT_in": np.ascontiguousarray(qs.T).astype(bf),
            "wkT_in": wkT,
            "wqT_in": wqT,
            "vP_in": vP,
        })

    global LAST_RESULT
    res = run_bass_kernel_spmd(
        nc, in_maps, core_ids=list(range(N_CORES)), trace=TRACE
    )
    LAST_RESULT = res
    ctx = np.concatenate([r["ctx_out"] for r in res.results], axis=0)[:, None, :]
    attn = np.concatenate([r["attn_out"] for r in res.results], axis=0)[:, None, :]
    return ctx, attn
